# revision 26
# baseline (speedup 1.0000x reference)
"""Matrix NMS (SOLOv2 gaussian decay) on 8 TRN2 NeuronCores.

Strategy: shard the pixel (h*w=40960) contraction dim across the 8 cores.
The host pre-casts the binary masks to fp8 (exact for 0/1), so each core
DMAs only 5.24MB (vs 21MB f32) and the PE consumes it directly with
DoubleRow fp8 matmuls.  Each core computes the partial Gram upper block
rows; block-row a drains as int16 straight into AllToAll shard a (one
contiguous [128, N-128a] write, ~2KB descriptors, copies alternating
DVE/ACT, queues alternating sync/scalar/gpsimd).  After the AllToAll each
core pulls the 8 partials of its own block-row (2KB-descriptor loads on 3
queues) and tree-sums them.  The epilogue is row-oriented: union comes
from a host-precomputed area table (areas are just mask sums — host
input), masked IoU d is formed with one approx-reciprocal pass, and the
two cross-candidate reductions (compensate_iou column max, final decay
max) are done as PE-chunk-transposes + DVE free-dim max, combined across
cores by two 4KB ReduceScatter(max) collectives.  Every core ends up
with the decayed scores for its own 128 candidates; the host concatenates
the 8 slices.
"""

import sys

import numpy as np

for _p in ("/opt/trn_rl_repo",):
    if _p not in sys.path:
        sys.path.insert(0, _p)

from concourse import bacc, bass, mybir, tile
from concourse import bass_utils

N = 1024           # candidates
HWPIX = 160 * 256  # 40960 pixels
W = 8              # cores
KC = HWPIX // W    # 5120 pixel-slice per core
KT = KC // 128     # 40 k-tiles of 128
GRP = 4            # k-tiles per resident SBUF group
RB = 128           # block-row height == shard rows
NP = KT // 2       # 20 k-tile pairs (DoubleRow)
SIGMA = 2.0

F32 = mybir.dt.float32
FP8 = mybir.dt.float8e4  # e4m3: exact for 0/1 mask values
I16 = mybir.dt.int16


def build_nc(variant="full"):
    # variant: "full" = real kernel; "nocc" = collectives replaced by local
    # DMA copies (wrong math, identical local compute/DMA — timing/sim only)
    nc = bacc.Bacc(
        "TRN2", target_bir_lowering=False, debug=False,
        num_devices=W if variant == "full" else 1,
    )

    xT = nc.dram_tensor("xT", [128, KT * N], FP8, kind="ExternalInput")
    maskR_h = nc.dram_tensor("maskR", [RB, N], F32, kind="ExternalInput")
    sjsc_h = nc.dram_tensor("sjsc", [RB, N], F32, kind="ExternalInput")
    scores_h = nc.dram_tensor("scores", [1, RB], F32, kind="ExternalInput")
    ident_h = nc.dram_tensor("ident", [128, 128], F32, kind="ExternalInput")
    out_h = nc.dram_tensor("out", [1, RB], F32, kind="ExternalOutput")

    RG = [list(range(W))]

    with tile.TileContext(nc) as tc:
        with (
            tc.tile_pool(name="dram", bufs=1, space="DRAM") as dramp,
            tc.tile_pool(name="xp", bufs=1) as xp,
            tc.tile_pool(name="pg", bufs=4, space="PSUM") as pgp,
            tc.tile_pool(name="gb", bufs=4) as gbp,
            tc.tile_pool(name="a2al", bufs=1) as alp,
            tc.tile_pool(name="sc", bufs=1) as scp,
            tc.tile_pool(name="epi", bufs=1) as ep,
        ):
            # AllToAll buffers split into column halves: the left half only
            # receives writes from block-rows 0-3, so it ships while blocks
            # 4-7 are still computing/draining.
            HN = N // 2
            cc_h = [dramp.tile([W * RB, HN], I16, tag=f"cc{x}", name=f"cc{x}")
                    for x in range(2)]
            a2a_h = [dramp.tile([W * RB, HN], I16, tag=f"a2a{x}", name=f"a2a{x}")
                     for x in range(2)]
            rs1_in = dramp.tile([1, N], F32, tag="rs1_in")
            rs1_out = dramp.tile([1, RB], F32, tag="rs1_out")
            rs2_in = dramp.tile([1, N], F32, tag="rs2_in")
            rs2_out = dramp.tile([1, RB], F32, tag="rs2_out")

            # constants + epilogue inputs (off the sync queue used by x loads)
            ident = scp.tile([128, 128], F32, tag="ident")
            nc.gpsimd.dma_start(ident[:], ident_h[:])
            scores = scp.tile([1, RB], F32, tag="scores")
            nc.gpsimd.dma_start(scores[:], scores_h[:])
            maskR = scp.tile([128, N], F32, tag="maskR")
            nc.scalar.dma_start(maskR[:], maskR_h[:])
            sjsc = scp.tile([128, N], F32, tag="sjsc")
            nc.scalar.dma_start(sjsc[:], sjsc_h[:])

            # ---- phase 1: fp8 x slice straight into SBUF (10 group loads)
            xg = [xp.tile([128, GRP, N], FP8, tag=f"x{g}", name=f"xg{g}")
                  for g in range(KT // GRP)]
            for g in range(KT // GRP):
                nc.sync.dma_start(xg[g][:], xT[:, g * GRP * N : (g + 1) * GRP * N])

            def xpair(q, c0, c1):
                t = 2 * q
                g, j = t // GRP, t % GRP
                return xg[g][:, j : j + 2, c0:c1]

            def gram_pair(pg, a, q):
                wdt = N - a * 128
                lhsT = xpair(q, a * 128, (a + 1) * 128)
                for off in range(0, wdt, 512):
                    cw = min(512, wdt - off)
                    nc.tensor.matmul(
                        pg[:, off : off + cw],
                        lhsT,
                        xpair(q, a * 128 + off, a * 128 + off + cw),
                        start=(q == 0),
                        stop=(q == NP - 1),
                        perf_mode=mybir.MatmulPerfMode.DoubleRow,
                    )

            def drain(a, pg):
                """PSUM block-row a -> int16 rows of AllToAll shard a.

                Shard a IS block-row a, so the write is one [128, N-128a]
                slab with per-partition-contiguous DRAM runs (up to 2KB).
                The left columns (<128a) of the shard stay garbage; the
                receiver's row mask zeroes them before any reduction.
                """
                wdt = N - a * 128
                gb16 = gbp.tile([128, wdt], I16, tag="gb16")
                if a % 2 == 0:
                    nc.vector.tensor_copy(gb16[:], pg[:, :wdt])
                else:
                    nc.scalar.activation(
                        gb16[:], pg[:, :wdt], mybir.ActivationFunctionType.Copy
                    )
                eng = (nc.sync, nc.scalar, nc.gpsimd)[a % 3]
                rows = slice(a * RB, (a + 1) * RB)
                if a * 128 < HN:
                    lw = HN - a * 128
                    eng.dma_start(cc_h[0][rows, a * 128 : HN], gb16[:, 0:lw])
                    eng.dma_start(cc_h[1][rows, :], gb16[:, lw:wdt])
                else:
                    eng.dma_start(cc_h[1][rows, a * 128 - HN : HN], gb16[:])

            # ---- AllToAll of the left column half (ready after wave A
            # drains); the right half ships after all drains, overlapping
            # the left half's receive-side loads and tree-sum.
            def a2a_chunk(x):
                if variant == "full":
                    nc.gpsimd.collective_compute(
                        "AllToAll",
                        mybir.AluOpType.bypass,
                        replica_groups=RG,
                        ins=[cc_h[x][:].opt()],
                        outs=[a2a_h[x][:].opt()],
                    )
                else:
                    nc.sync.dma_start(a2a_h[x][:], cc_h[x][:])

            # ---- local tree-sum of the 8 partials of this core's block-row
            # (per half: 4 pair-loads on alternating queues, 1KB descriptors)
            summ_h = [ep.tile([128, HN], F32, tag=f"summ{x}", name=f"summ{x}")
                      for x in range(2)]

            def tree_half(x):
                lv = []
                for h in range(4):
                    lt = alp.tile([RB, 2, HN], I16, tag=f"ld{x}{h}", name=f"ld{x}{h}")
                    src = a2a_h[x][2 * h * RB : (2 * h + 2) * RB, :].rearrange(
                        "(s p) n -> p s n", p=RB
                    )
                    eng = (nc.sync, nc.scalar, nc.gpsimd, nc.sync)[h]
                    eng.dma_start(lt[:], src)
                    lv.append(lt)
                m01 = ep.tile([RB, 2, HN], I16, tag=f"m01{x}")
                nc.vector.tensor_add(m01[:], lv[0][:], lv[1][:])
                m23 = ep.tile([RB, 2, HN], I16, tag=f"m23{x}")
                nc.vector.tensor_add(m23[:], lv[2][:], lv[3][:])
                p2 = ep.tile([RB, 2, HN], I16, tag=f"p2{x}")
                nc.vector.tensor_add(p2[:], m01[:], m23[:])
                nc.vector.tensor_add(summ_h[x][:], p2[:, 0, :], p2[:, 1, :])

            # ---- phase 2: Gram upper block-rows in two PSUM waves
            wave_a = [pgp.tile([128, N - a * 128], F32, tag="pg", name=f"pgA{a}")
                      for a in range(4)]
            for q in range(NP):
                for a in range(4):
                    gram_pair(wave_a[a], a, q)
            for a in range(4):
                drain(a, wave_a[a])
            a2a_chunk(0)
            for a in range(4, W):
                pg = pgp.tile([128, N - a * 128], F32, tag="pg")
                for q in range(NP):
                    gram_pair(pg, a, q)
                drain(a, pg)
            a2a_chunk(1)

            # ---- row-oriented epilogue, column-separable up to the
            # column max: the left half runs while the right A2A chunk flies
            f = ep.tile([128, N], F32, tag="f")

            def col_max_half(srcx, x, name, dst):
                """[128, HN] -> column max row written into dst (global col
                order) via PE chunk transposes + DVE free-dim reduce."""
                tp = pgp.tile([128, 4, 128], F32, tag="pg", name=f"{name}{x}_tp")
                for k in range(4):
                    nc.tensor.transpose(
                        tp[:, k, :], srcx[:, k * 128 : (k + 1) * 128], ident[:]
                    )
                mx4 = ep.tile([128, 4], F32, tag=f"{name}{x}_mx4")
                nc.vector.tensor_reduce(
                    mx4[:], tp[:], axis=mybir.AxisListType.X, op=mybir.AluOpType.max
                )
                m4ps = pgp.tile([4, 128], F32, tag="pg", name=f"{name}{x}_m4ps")
                nc.tensor.transpose(m4ps[:], mx4[:], ident[:])
                m4s = ep.tile([4, 128], F32, tag=f"{name}{x}_m4s")
                nc.vector.tensor_copy(m4s[:], m4ps[:])
                nc.sync.dma_start(dst, m4s[:])

            def epi_half(x):
                cols = slice(x * HN, (x + 1) * HN)
                un = ep.tile([128, HN], F32, tag=f"un{x}", name=f"un{x}")
                nc.vector.tensor_tensor(
                    un[:], sjsc[:, cols], summ_h[x][:], op=mybir.AluOpType.subtract
                )
                nc.vector.tensor_scalar(
                    un[:], un[:], 1.0, None, op0=mybir.AluOpType.max
                )
                rec = ep.tile([128, HN], F32, tag=f"rec{x}", name=f"rec{x}")
                nc.vector.reciprocal_approx_fast(rec[:], un[:])
                dx = ep.tile([128, HN], F32, tag=f"d{x}", name=f"d{x}")
                nc.vector.tensor_mul(dx[:], summ_h[x][:], rec[:])
                nc.vector.tensor_mul(dx[:], dx[:], maskR[:, cols])
                # partial compensate_iou for this half -> its rs1_in slice
                col_max_half(dx, x, "pc", rs1_in[0:1, cols])
                # d^2 feeds the second reduction; fills f while CCs fly
                nc.vector.tensor_mul(f[:, cols], dx[:], dx[:])

            tree_half(0)
            epi_half(0)
            tree_half(1)
            epi_half(1)

            if variant == "full":
                nc.gpsimd.collective_compute(
                    "ReduceScatter",
                    mybir.AluOpType.max,
                    replica_groups=RG,
                    ins=[rs1_in[:].opt()],
                    outs=[rs1_out[:].opt()],
                )
            else:
                nc.sync.dma_start(rs1_out[:], rs1_in[:, 0:RB])
            crow = ep.tile([1, RB], F32, tag="crow")
            nc.sync.dma_start(crow[:], rs1_out[:])
            one1 = ep.tile([1, 1], F32, tag="one1")
            nc.vector.memset(one1[:], 1.0)
            # crow.T via k=1 matmul: out[128,1] = crow[1,128].T @ ones[1,1]
            c_ps = pgp.tile([128, 1], F32, tag="pg", name="c_ps")
            nc.tensor.matmul(c_ps[:], crow[:], one1[:], start=True, stop=True)
            c_own = ep.tile([128, 1], F32, tag="c_own")
            nc.vector.tensor_copy(c_own[:], c_ps[:])
            c2 = ep.tile([128, 1], F32, tag="c2")
            nc.vector.tensor_mul(c2[:], c_own[:], c_own[:])
            # f[p, j] = d^2 - c_own^2 ; M[j] = global max of f over rows
            nc.vector.tensor_scalar(
                f[:], f[:], c2[:], None, op0=mybir.AluOpType.subtract
            )
            for x in range(2):
                col_max_half(
                    f[:, x * HN : (x + 1) * HN], x, "pm",
                    rs2_in[0:1, x * HN : (x + 1) * HN],
                )
            if variant == "full":
                nc.gpsimd.collective_compute(
                    "ReduceScatter",
                    mybir.AluOpType.max,
                    replica_groups=RG,
                    ins=[rs2_in[:].opt()],
                    outs=[rs2_out[:].opt()],
                )
            else:
                nc.sync.dma_start(rs2_out[:], rs2_in[:, 0:RB])
            m_red = ep.tile([1, RB], F32, tag="m_red")
            nc.sync.dma_start(m_red[:], rs2_out[:])
            # out = scores * exp(-sigma * M) for this core's 128 candidates
            coeff = ep.tile([1, RB], F32, tag="coeff")
            nc.scalar.activation(
                coeff[:], m_red[:], mybir.ActivationFunctionType.Exp, scale=-SIGMA
            )
            outsb = ep.tile([1, RB], F32, tag="outsb")
            nc.vector.tensor_mul(outsb[:], coeff[:], scores[:])
            nc.scalar.dma_start(out_h[:], outsb[:])

    nc.compile()
    return nc


_NC_CACHE = {}


def _get_nc(variant="full"):
    if variant not in _NC_CACHE:
        _NC_CACHE[variant] = build_nc(variant)
    return _NC_CACHE[variant]


def make_in_maps(seg_masks, cate_labels, cate_scores):
    import ml_dtypes

    flat = np.asarray(seg_masks, dtype=np.float32).reshape(N, -1)
    labels = np.asarray(cate_labels)
    scores = np.asarray(cate_scores, dtype=np.float32)
    areas = flat.sum(axis=1)  # exact integers in f32
    xTfull = np.ascontiguousarray(flat.T)  # (40960, 1024)
    gidx = np.arange(N)
    ident = np.eye(128, dtype=np.float32)
    in_maps = []
    for c in range(W):
        rows = slice(c * RB, (c + 1) * RB)
        gr = gidx[rows]
        maskR = (
            (gidx[None, :] > gr[:, None]) & (labels[None, :] == labels[rows][:, None])
        ).astype(np.float32)
        sjsc = areas[None, :] + areas[rows][:, None]  # s_j + s_i, (128, N)
        in_maps.append(
            {
                # partition-major: row p holds k-rows {p, 128+p, ...} of this
                # core's slice; host casts to fp8 (exact for 0/1 masks)
                "xT": np.ascontiguousarray(
                    xTfull[c * KC : (c + 1) * KC]
                    .reshape(KT, 128, N)
                    .transpose(1, 0, 2)
                ).reshape(128, KT * N).astype(ml_dtypes.float8_e4m3fn),
                "maskR": maskR,
                "sjsc": np.ascontiguousarray(sjsc, dtype=np.float32),
                "scores": np.ascontiguousarray(scores[rows].reshape(1, RB)),
                "ident": ident,
            }
        )
    return in_maps


def run_device(in_maps, trace=False):
    nc = _get_nc()
    res = bass_utils.run_bass_kernel_spmd(
        nc, in_maps, core_ids=list(range(W)), trace=trace
    )
    return res


def kernel(seg_masks, cate_labels, cate_scores):
    in_maps = make_in_maps(seg_masks, cate_labels, cate_scores)
    res = run_device(in_maps)
    outs = [np.asarray(res.results[c]["out"]).reshape(RB) for c in range(W)]
    return np.concatenate(outs).astype(np.float32)
